# revision 5
# baseline (speedup 1.0000x reference)
"""Trainium2 Bass kernel for nn_BertClassifier_77309411685 (V8).

Data-parallel over 8 NeuronCores: each core handles 256 samples; the small
base linear and 12 expert heads are replicated.

V8 strategy (from V7 trace forensics: weight DMA trickled 8.7->33us on
512-B strided descriptors, PE spent ~10us on diag-stationary masked-mean
matmuls, first matmul waited on a const DMA stuck behind ACT_TABLE_LOAD):
  * fp16 end-to-end (host-cast); accumulations in fp32/fp16 mix.
  * samples per core are permuted by span length (host-side; output is
    un-permuted on host): group A = 128 shortest spans (gather JA rows),
    group B = 128 longest (JB rows).  Cuts gather bytes ~20%.
  * span gather: per group, TWO indirect DMAs (rows [0,J1) and [J1,J)),
    each one contiguous descriptor per sample.  Single-index offset APs
    only (the HW DGE misreads multi-index offset APs).
  * masked mean on Vector+GpSimd via fused scalar_tensor_tensor chains
    (acc = g_j * m_j + acc, fp16, masks pre-scaled by 1/len) - frees the
    PE entirely from the mean.
  * wbT is shipped HOST-PREARRANGED in the [128, KC*INNER] SBUF layout so
    every weight DMA is 128 x multi-KB contiguous descriptors.
  * consts ride the Vector queue (Scalar's queue stalls behind its
    ACT_TABLE_LOAD); gidx rides GpSimd (same-engine completion tracking).
  * PE warm-up feeds from a memset tile - no DMA dependency, so the array
    warms during the DMA ramp.
  * base linear: ctx k-chunks run as weight/ctxT chunk pairs land; center
    k-chunks close the PSUM accumulation; bias+relu fused.
  * expert heads: all 12 experts at once with bias folded via a ones row;
    per-sample selection by is_equal mask + strided reduce; one packed
    [128, 6] output DMA at the very end.
"""

import numpy as np
from contextlib import ExitStack

import concourse.bass as bass
import concourse.tile as tile
from concourse import bacc, mybir
from concourse.bass import IndirectOffsetOnAxis
from concourse.bass_utils import run_bass_kernel_spmd

F32 = mybir.dt.float32
F16 = mybir.dt.float16
I32 = mybir.dt.int32

B, S, H = 2048, 256, 768
INNER, NB_CTX, NB_EXPERTS, NB_LABELS = 256, 2, 12, 3
NCORES = 8
BC = B // NCORES             # 256 samples per core
F3H = (NB_CTX + 1) * H       # 2304
KC = F3H // 128              # 18 contraction chunks
HC = H // 128                # 6 chunks per feature block
NE = NB_EXPERTS * NB_LABELS  # 36
EROWS = BC * S               # rows in the per-core embedding tensor

# The reference picks 2 static context positions host-side with this exact rng.
CTX_IDX = [int(v) for v in np.random.default_rng(seed=0).choice(np.arange(S), size=NB_CTX)]

MUL = mybir.AluOpType.mult
ADD = mybir.AluOpType.add


def _build(JA, JB):
    """Build the per-core program for group row counts (JA, JB)."""
    J1 = [(JA + 1) // 2, (JB + 1) // 2]          # gpsimd piece rows
    J2 = [JA - J1[0], JB - J1[1]]                # vector piece rows
    JS = [JA, JB]
    MOFF = NE + 2                                # mask cols offset in c32

    nc = bacc.Bacc(
        "TRN2",
        target_bir_lowering=False,
        debug=False,
        enable_asserts=False,
        num_devices=NCORES,
    )
    embT = nc.dram_tensor("embT", [EROWS, H], F16, kind="ExternalInput").ap()
    gidx = nc.dram_tensor("gidx", [128, 4], I32, kind="ExternalInput").ap()
    # wbT pre-arranged: wbT[p, c*INNER + m] = W_base[m, c*128 + p]
    wbT = nc.dram_tensor("wbT", [128, KC * INNER], F16, kind="ExternalInput").ap()
    ctxT = nc.dram_tensor("ctxT", [128, NB_CTX * HC * 256], F16, kind="ExternalInput").ap()
    # c16: identity [0:128) + wexpA [128:164) + wexpB [164:200)
    c16 = nc.dram_tensor("c16", [128, 128 + 2 * NE], F16, kind="ExternalInput").ap()
    # c32: io36 [0:36) + categories-as-float [36:38) + masks/len [38:38+JA+JB)
    #      + b_base (t p) layout (last 2)
    c32 = nc.dram_tensor("c32", [128, MOFF + JA + JB + 2], F32, kind="ExternalInput").ap()
    # c1: ones row [0:256) + expert bias row [256:292)
    c1 = nc.dram_tensor("c1", [1, 256 + NE], F16, kind="ExternalInput").ap()
    out = nc.dram_tensor("out", [128, 2 * NB_LABELS], F32, kind="ExternalOutput").ap()

    with tile.TileContext(nc) as tc, ExitStack() as ctx:
        pool = ctx.enter_context(tc.tile_pool(name="main", bufs=1))
        pst = ctx.enter_context(tc.tile_pool(name="pst", bufs=1, space="PSUM"))

        # --- tiny front-of-queue loads ---
        # gidx rides the gpsimd (SWDGE) queue: same-engine completion
        # tracking for the gather descriptor generation that reads it.
        gidx_t = pool.tile([128, 4], I32)
        nc.gpsimd.dma_start(gidx_t[:], gidx[:, :])

        # consts ride the Vector queue (Scalar's queue stalls behind the
        # auto-hoisted ACT_TABLE_LOAD).
        c16_t = pool.tile([128, 128 + 2 * NE], F16)
        nc.scalar.dma_start(c16_t[:], c16[:, :])
        identity = c16_t[:, 0:128]
        wexpA = c16_t[:, 128:128 + NE]
        wexpB = c16_t[:, 128 + NE:128 + 2 * NE]
        c32_t = pool.tile([128, MOFF + JA + JB + 2], F32)
        nc.scalar.dma_start(c32_t[:], c32[:, :])
        io36f = c32_t[:, 0:NE]
        catf = c32_t[:, NE:NE + 2]
        bb_t = c32_t[:, MOFF + JA + JB:MOFF + JA + JB + 2]
        c1_t = pool.tile([1, 256 + NE], F16)
        nc.scalar.dma_start(c1_t[:], c1[:, :])
        ones1 = c1_t[:, 0:256]
        wexpC = c1_t[:, 256:256 + NE]

        # PE warm-up from a memset tile (no DMA dependency): the HAM clock
        # gate needs ~3.4us of sustained activity; run it during the DMA ramp.
        warm_src = pool.tile([128, 512], F16)
        nc.vector.memset(warm_src[:], 0.0)
        warm = pst.tile([128, 512], F32, tag="warm", bufs=1)
        for w in range(6):
            nc.tensor.matmul(warm[:], lhsT=warm_src[:, 0:128], rhs=warm_src[:],
                             start=(w == 0), stop=(w == 5))

        # --- span gathers: 2 pieces per group, one contiguous descriptor
        # per sample per piece ---
        g_piece = [[None, None], [None, None]]
        for g in range(2):
            for pc in range(2):
                rows = (J1[g], J2[g])[pc]
                if rows == 0:
                    continue
                gt = pool.tile([128, rows * H], F16, tag=f"g{g}{pc}", bufs=1)
                nc.gpsimd.indirect_dma_start(
                    out=gt[:], out_offset=None, in_=embT,
                    in_offset=IndirectOffsetOnAxis(
                        ap=gidx_t[:, 2 * g + pc:2 * g + pc + 1], axis=0),
                )
                g_piece[g][pc] = gt

        # --- weight streams on the Sync queue, interleaved so ctx-chunk
        # matmuls start as soon as each (wbT, ctxT) chunk pair lands ---
        featT = pool.tile([128, KC * 256], F16)
        wbT_t = pool.tile([128, KC * INNER], F16)
        nc.sync.dma_start(wbT_t[:, HC * INNER:2 * HC * INNER],
                          wbT[:, HC * INNER:2 * HC * INNER])
        nc.sync.dma_start(featT[:, HC * 256:2 * HC * 256],
                          ctxT[:, 0:HC * 256])
        nc.sync.dma_start(wbT_t[:, 2 * HC * INNER:],
                          wbT[:, 2 * HC * INNER:])
        nc.sync.dma_start(featT[:, 2 * HC * 256:],
                          ctxT[:, HC * 256:])
        nc.sync.dma_start(wbT_t[:, :HC * INNER], wbT[:, :HC * INNER])

        # --- base linear ctx chunks: open the 4 PSUM accumulation groups ---
        accs = [[pst.tile([128, 128], F32, tag=f"acc{g}{mt}", bufs=1,
                          name=f"acc{g}{mt}") for mt in range(2)]
                for g in range(2)]
        for c in range(HC, KC):
            for g in range(2):
                for mt in range(2):
                    nc.tensor.matmul(
                        accs[g][mt][:],
                        lhsT=wbT_t[:, c * INNER + mt * 128: c * INNER + (mt + 1) * 128],
                        rhs=featT[:, c * 256 + g * 128: c * 256 + (g + 1) * 128],
                        start=(c == HC), stop=False,
                    )

        # --- masked mean: fused mult-add chain per group on Vector
        # (scalar_tensor_tensor is Vector-only), split across the two gather
        # pieces so the chain pipelines with the DMA stream ---
        ct_g = []
        for g in range(2):
            ct = pool.tile([128, H], F16, name=f"ct{g}")
            j = 0
            for pc in range(2):
                gt = g_piece[g][pc]
                if gt is None:
                    continue
                rows = (J1[g], J2[g])[pc]
                for t in range(rows):
                    sc = c32_t[:, MOFF + g * JA + j: MOFF + g * JA + j + 1]
                    src = gt[:, t * H:(t + 1) * H]
                    if j == 0:
                        nc.vector.tensor_scalar(ct[:], src, sc, None, op0=MUL)
                    else:
                        nc.vector.scalar_tensor_tensor(ct[:], src, sc, ct[:],
                                                       op0=MUL, op1=ADD)
                    j += 1
            ct_g.append(ct)

        featT_pairs = featT[:].rearrange("p (c x) -> p c x", x=256)
        hiddenT = pool.tile([128, 2 * 256], F16)
        out3 = pool.tile([128, 2 * NB_LABELS], F32)  # [p, g*3 + n]
        ps36 = pst.tile([128, 2 * NE], F32, tag="e36", bufs=1)

        for g in range(2):
            ct = ct_g[g]
            # center transposes (PE transpose mode, identity permutation)
            tpc = pst.tile([128, HC * 128], F16, tag=f"tpc{g}", bufs=1)
            for cc in range(HC):
                nc.tensor.transpose(tpc[:, cc * 128:(cc + 1) * 128],
                                    ct[:, cc * 128:(cc + 1) * 128], identity)
            # drain PSUM -> featT split across Scalar and Vector (GpSimd
            # cannot access PSUM)
            tpcv = tpc[:].rearrange("p (c x) -> p c x", c=HC)
            nc.scalar.copy(featT_pairs[:, 0:3, g * 128:(g + 1) * 128],
                           tpcv[:, 0:3, :])
            nc.vector.tensor_copy(featT_pairs[:, 3:HC, g * 128:(g + 1) * 128],
                                  tpcv[:, 3:HC, :])

            # center chunks close the base-linear accumulation; bias+relu
            for c in range(HC):
                for mt in range(2):
                    nc.tensor.matmul(
                        accs[g][mt][:],
                        lhsT=wbT_t[:, c * INNER + mt * 128: c * INNER + (mt + 1) * 128],
                        rhs=featT[:, c * 256 + g * 128: c * 256 + (g + 1) * 128],
                        start=False, stop=(c == HC - 1),
                    )
            for mt in range(2):
                nc.scalar.activation(
                    hiddenT[:, mt * 256 + g * 128: mt * 256 + (g + 1) * 128],
                    accs[g][mt][:],
                    mybir.ActivationFunctionType.Relu,
                    bias=bb_t[:, mt:mt + 1], scale=1.0)

        for g in range(2):
            # expert heads + per-sample selection
            b0 = g * 128
            mask36 = pool.tile([128, NE], F32, tag=f"mask36{g}", bufs=1)
            nc.vector.tensor_scalar(mask36[:], io36f, catf[:, g:g + 1], None,
                                    op0=mybir.AluOpType.is_equal)
            pe = ps36[:, g * NE:(g + 1) * NE]
            nc.tensor.matmul(pe, lhsT=hiddenT[:, b0:b0 + 128],
                             rhs=wexpA, start=True, stop=False)
            nc.tensor.matmul(pe, lhsT=hiddenT[:, 256 + b0:256 + b0 + 128],
                             rhs=wexpB, start=False, stop=False)
            nc.tensor.matmul(pe, lhsT=ones1[0:1, b0:b0 + 128],
                             rhs=wexpC, start=False, stop=True)

            prod = pool.tile([128, NE], F32, tag=f"prod{g}", bufs=1)
            nc.vector.tensor_tensor(out=prod[:], in0=pe, in1=mask36[:], op=MUL)
            nc.vector.tensor_reduce(
                out=out3[:, g * NB_LABELS:(g + 1) * NB_LABELS],
                in_=prod[:].rearrange("p (e n) -> p n e", n=NB_LABELS),
                axis=mybir.AxisListType.X, op=ADD)

        nc.sync.dma_start(out[:, :], out3[:])

    nc.compile()
    return nc


_NC = {}


def _get_nc(JA, JB):
    key = (JA, JB)
    if key not in _NC:
        _NC[key] = _build(JA, JB)
    return _NC[key]


def _prep_inputs(embeddings, position_indexes, categories, W_base, b_base,
                 W_experts, b_experts):
    emb32 = np.asarray(embeddings)
    emb16 = emb32.astype(np.float16).reshape(NCORES, BC, S, H)

    pos = np.asarray(position_indexes).astype(np.int64).reshape(NCORES, BC, 2)
    cat = np.asarray(categories).astype(np.int64).reshape(NCORES, BC)

    lens_all = pos[:, :, 1] - pos[:, :, 0]                     # [NC, 256]
    perm = np.argsort(lens_all, axis=1, kind="stable")         # [NC, 256]
    lensP = np.take_along_axis(lens_all, perm, 1)
    startsP = np.take_along_axis(pos[:, :, 0], perm, 1)
    catP = np.take_along_axis(cat, perm, 1)

    JA = int(lensP[:, :128].max())
    JB = int(lensP[:, 128:].max())
    assert 1 <= JA <= 8 and 1 <= JB <= 8
    J1 = [(JA + 1) // 2, (JB + 1) // 2]

    # gather start rows [128, 4]: (A piece0, A piece1, B piece0, B piece1)
    row = perm * S + startsP                                   # [NC, 256]
    gidx = np.empty((NCORES, 128, 4), dtype=np.int32)
    gidx[:, :, 0] = row[:, :128]
    gidx[:, :, 1] = row[:, :128] + J1[0]
    gidx[:, :, 2] = row[:, 128:]
    gidx[:, :, 3] = row[:, 128:] + J1[1]

    # base linear: wbT[p, c*INNER+m] = W_base[m, c*128+p], shipped contiguous
    wb = np.asarray(W_base, dtype=np.float32)  # [INNER, 3H]
    wbT = np.ascontiguousarray(
        wb.T.reshape(KC, 128, INNER).transpose(1, 0, 2).reshape(128, KC * INNER)
    ).astype(np.float16)

    bbias = np.asarray(b_base, dtype=np.float32)

    we = np.asarray(W_experts, dtype=np.float32)  # [12, 3, INNER]
    be = np.asarray(b_experts, dtype=np.float32)  # [12, 3]
    wexp = we.transpose(2, 0, 1).reshape(INNER, NE)
    eye = np.eye(128, dtype=np.float16)
    c16 = np.concatenate(
        [np.broadcast_to(eye[None], (NCORES, 128, 128)),
         np.broadcast_to(wexp[None, 0:128].astype(np.float16), (NCORES, 128, NE)),
         np.broadcast_to(wexp[None, 128:256].astype(np.float16), (NCORES, 128, NE))],
        axis=2)
    c1 = np.concatenate(
        [np.ones((1, 256), dtype=np.float32), be.reshape(1, NE)],
        axis=1).astype(np.float16)

    # static context rows in featT layout, permuted order:
    # ctxT[p, (which*HC+cc)*256 + g*128 + sl] = emb[perm[g*128+sl], CTX_IDX[which], cc*128+p]
    blocks = []
    for which in range(NB_CTX):
        blk = emb16[:, :, CTX_IDX[which], :]                   # [NC, 256, 768]
        blkP = np.take_along_axis(blk, perm[:, :, None], 1)
        arr = blkP.reshape(NCORES, 2, 128, HC, 128).transpose(0, 4, 3, 1, 2)
        blocks.append(arr.reshape(NCORES, 128, HC * 256))
    ctxT = np.ascontiguousarray(np.concatenate(blocks, axis=2))

    # io36 + categories-as-float + masks(1/len scaled) + b_base
    MOFF = NE + 2
    cst32 = np.zeros((NCORES, 128, MOFF + JA + JB + 2), dtype=np.float32)
    cst32[:, :, :NE] = np.repeat(np.arange(NB_EXPERTS, dtype=np.float32),
                                 NB_LABELS)[None, None, :]
    cst32[:, :, NE:NE + 2] = catP.reshape(NCORES, 2, 128).transpose(0, 2, 1)
    for g, J in ((0, JA), (1, JB)):
        lens_g = lensP[:, g * 128:(g + 1) * 128].astype(np.float32)  # [NC, 128]
        j = np.arange(J, dtype=np.float32)
        m = (j[None, None, :] < lens_g[:, :, None]) / lens_g[:, :, None]
        off = MOFF + g * JA
        cst32[:, :, off:off + J] = m
    cst32[:, :, MOFF + JA + JB:] = bbias.reshape(2, 128).T[None]

    in_maps = [
        {"embT": np.ascontiguousarray(emb16[i].reshape(EROWS, H)),
         "gidx": np.ascontiguousarray(gidx[i]),
         "wbT": wbT, "ctxT": ctxT[i],
         "c16": np.ascontiguousarray(c16[i]),
         "c32": np.ascontiguousarray(cst32[i]),
         "c1": np.ascontiguousarray(c1)}
        for i in range(NCORES)
    ]
    return {"in_maps": in_maps, "perm": perm, "key": (JA, JB)}


def _run(prep, **kw):
    nc = _get_nc(*prep["key"])
    return run_bass_kernel_spmd(nc, prep["in_maps"],
                                core_ids=list(range(NCORES)), **kw)


def _postprocess(prep, res):
    perm = prep["perm"]
    full = np.empty((B, NB_LABELS), dtype=np.float32)
    for i, r in enumerate(res.results):
        arr = r["out"].reshape(128, 2, NB_LABELS).transpose(1, 0, 2).reshape(
            BC, NB_LABELS)
        full[i * BC + perm[i]] = arr
    return full


def kernel(embeddings, position_indexes, categories, W_base, b_base, W_experts,
           b_experts):
    prep = _prep_inputs(embeddings, position_indexes, categories, W_base,
                        b_base, W_experts, b_experts)
    res = _run(prep)
    return _postprocess(prep, res)
